# revision 6
# baseline (speedup 1.0000x reference)
"""Trainium2 (8 NeuronCores) kernel for nn_AdaptiveSliceSelector.

Strategy:
- Algebraic refold: GAT aggregation commutes with the per-branch weight
  matrix, so per-edge work happens in x-space (256-wide rows) and the
  weight matmuls (Wg@Ws folded into one 256x256 per branch) run on the
  aggregated output. Attention logits use folded vectors p=Wg@a_src,
  q=Wg@a_dst:  s = x@p, d = x@q,  w_e = exp(leaky_relu(s[src]+d[dst])).
  exp() without the max-shift is safe (logits ~ N(0,2)) and cancels in
  the softmax ratio.
- Self-loops are appended as ordinary edges on the host.
- Nodes are sharded round-robin-free: core c owns rows [c*R, (c+1)*R).
  The bf16 gather table (x rows + a validity flag column) is replicated
  to every core's HBM, so cross-partition edges need no halo exchange;
  per-edge rows are fetched with indirect DMA by global src index.
- Per (dst-tile, branch): gathered edge rows [128e, 272] become the
  matmul moving operand; the one-hot scatter matrix S[e, j] =
  w_e * (dstloc_e == j) is built on DVE/ACT and used as lhsT so the
  segment-softmax numerator/denominator come out of one PSUM matmul
  accumulation (the flag column yields the denominator, and host-side
  padding edges point at a zero row so they vanish from both).
- Strategy weights (softmax MLP over mean(x)) use a 1KB AllReduce.
"""

import os
import sys

sys.path.insert(0, "/opt/trn_rl_repo")

import numpy as np
import ml_dtypes

import concourse.bass as bass
import concourse.mybir as mybir
import concourse.tile as tile
from concourse import bacc
from concourse.bass_utils import run_bass_kernel_spmd

NCORES = 8
H = 256
P = 128
GROW = 272  # gather-table row width (bf16) -> 544B rows, 32B aligned
FLAG = H    # column holding the 1.0 validity flag
LN_EPS = 1e-5
NEG_SLOPE = 0.2

F32 = mybir.dt.float32
BF16 = mybir.dt.bfloat16
I32 = mybir.dt.int32
AO = mybir.AluOpType
AF = mybir.ActivationFunctionType

LAST_EXEC_NS = None  # stash for test harness
LAST_RES = None

_GRAPH_CACHE = {}


def _bf(a):
    return np.asarray(a, np.float32).astype(ml_dtypes.bfloat16)


# --------------------------------------------------------------------------
# device graph
# --------------------------------------------------------------------------

def _build(T, GR, cgs, NREAL):
    """Build the SPMD Bass graph.

    T: dst tiles per core; GR: gather table rows; cgs: list of chunk
    counts in (t, b) order, len == 3*T.
    """
    R = T * P
    totc = int(sum(cgs))

    nc = bacc.Bacc("TRN2", target_bir_lowering=False, debug=False,
                   num_devices=NCORES)

    gx = nc.dram_tensor("gx", [GR, GROW], BF16, kind="ExternalInput")
    xt_in = nc.dram_tensor("xt", [P, 2 * R], BF16, kind="ExternalInput")
    wc_in = nc.dram_tensor("wc", [P, 5 * 2 * H], BF16, kind="ExternalInput")
    prep_in = nc.dram_tensor("prep", [P, 3 * H], BF16, kind="ExternalInput")
    pq_in = nc.dram_tensor("pq", [P, 2 * 6], BF16, kind="ExternalInput")
    gsb_in = nc.dram_tensor("gsb", [P, 4 * H], F32, kind="ExternalInput")
    btb_in = nc.dram_tensor("btb", [P, 4 * H], F32, kind="ExternalInput")
    cbb_in = nc.dram_tensor("cbb", [P, 4 * H], F32, kind="ExternalInput")
    fin_in = nc.dram_tensor("fin", [P, 3 * H], F32, kind="ExternalInput")
    w1_in = nc.dram_tensor("w1", [P, 2 * P], F32, kind="ExternalInput")
    b1_in = nc.dram_tensor("b1", [P, 1], F32, kind="ExternalInput")
    w2_in = nc.dram_tensor("w2", [P, 4], F32, kind="ExternalInput")
    b2_in = nc.dram_tensor("b2", [P, 1], F32, kind="ExternalInput")
    aux_in = nc.dram_tensor("aux", [P, 2 * P], F32, kind="ExternalInput")
    auxb_in = nc.dram_tensor("auxb", [P, P], BF16, kind="ExternalInput")
    srcT_in = nc.dram_tensor("srcT", [P, totc], I32, kind="ExternalInput")
    dstT_in = nc.dram_tensor("dstT", [P, totc], F32, kind="ExternalInput")
    out = nc.dram_tensor("out", [R, H], F32, kind="ExternalOutput")

    with tile.TileContext(nc) as tc:
        with (
            tc.tile_pool(name="const", bufs=1) as cpool,
            tc.tile_pool(name="work", bufs=3) as work,
            tc.tile_pool(name="epi", bufs=2) as epi,
            tc.tile_pool(name="psum", bufs=2, space="PSUM") as psum,
            tc.tile_pool(name="psy", bufs=2, space="PSUM") as psy,
            tc.tile_pool(name="dram", bufs=1, space="DRAM") as dpool,
        ):
            # ---- resident loads ----
            xt = cpool.tile([P, 2, R], BF16)
            nc.sync.dma_start(out=xt[:], in_=xt_in.ap())
            wc = cpool.tile([P, 5, 2, H], BF16)
            nc.sync.dma_start(out=wc[:], in_=wc_in.ap())
            prep = cpool.tile([P, 3, H], BF16)
            nc.sync.dma_start(out=prep[:], in_=prep_in.ap())
            pq = cpool.tile([P, 2, 6], BF16)
            nc.sync.dma_start(out=pq[:], in_=pq_in.ap())
            gsb = cpool.tile([P, 4, H], F32)
            nc.sync.dma_start(out=gsb[:], in_=gsb_in.ap())
            btb = cpool.tile([P, 4, H], F32)
            nc.sync.dma_start(out=btb[:], in_=btb_in.ap())
            cbb = cpool.tile([P, 4, H], F32)
            nc.sync.dma_start(out=cbb[:], in_=cbb_in.ap())
            fin = cpool.tile([P, 3, H], F32)
            nc.sync.dma_start(out=fin[:], in_=fin_in.ap())
            w1 = cpool.tile([P, 2, P], F32)
            nc.sync.dma_start(out=w1[:], in_=w1_in.ap())
            b1 = cpool.tile([P, 1], F32)
            nc.sync.dma_start(out=b1[:], in_=b1_in.ap())
            w2 = cpool.tile([P, 4], F32)
            nc.sync.dma_start(out=w2[:], in_=w2_in.ap())
            b2 = cpool.tile([P, 1], F32)
            nc.sync.dma_start(out=b2[:], in_=b2_in.ap())
            aux = cpool.tile([P, 2, P], F32)
            nc.sync.dma_start(out=aux[:], in_=aux_in.ap())
            idf = aux[:, 0, :]
            iota = aux[:, 1, :]
            idb = cpool.tile([P, P], BF16)
            nc.sync.dma_start(out=idb[:], in_=auxb_in.ap())
            epsc = cpool.tile([P, 1], F32)
            nc.gpsimd.memset(epsc[:], LN_EPS)
            srcT = cpool.tile([P, totc], I32)
            nc.sync.dma_start(out=srcT[:], in_=srcT_in.ap())
            dstT = cpool.tile([P, totc], F32)
            nc.sync.dma_start(out=dstT[:], in_=dstT_in.ap())

            # ---- strategy weights: colsum -> AllReduce -> MLP -> sw ----
            cs = cpool.tile([P, 2], F32)
            nc.vector.tensor_reduce(out=cs[:], in_=xt[:],
                                    axis=mybir.AxisListType.X, op=AO.add)
            cin = dpool.tile([P, 2], F32)
            cout = dpool.tile([P, 2], F32)
            nc.gpsimd.dma_start(out=cin[:], in_=cs[:])
            nc.gpsimd.collective_compute(
                "AllReduce", AO.add,
                ins=[cin.opt()], outs=[cout.opt()],
                replica_groups=[list(range(NCORES))],
            )
            gsum = cpool.tile([P, 2], F32)
            nc.gpsimd.dma_start(out=gsum[:], in_=cout[:])
            gmean = cpool.tile([P, 2], F32)
            nc.vector.tensor_scalar_mul(gmean[:], gsum[:], 1.0 / NREAL)

            hps = psum.tile([P, 1], F32, tag="drp")
            for k in range(2):
                nc.tensor.matmul(hps[:], lhsT=w1[:, k, :], rhs=gmean[:, k:k + 1],
                                 start=(k == 0), stop=(k == 1))
            hsb = cpool.tile([P, 1], F32)
            nc.scalar.activation(hsb[:], hps[:], AF.Relu, bias=b1[:])
            lps = psum.tile([P, 4], F32, tag="drp")
            nc.tensor.matmul(lps[:4, :1], lhsT=w2[:], rhs=hsb[:],
                             start=True, stop=True)
            lsb = cpool.tile([P, 1], F32)
            nc.vector.tensor_tensor(out=lsb[:4, :], in0=lps[:4, :1],
                                    in1=b2[:4, :], op=AO.add)
            # transpose+replicate the 4 logits to all partitions
            lrp = psum.tile([P, 4], F32, tag="drp")
            nc.tensor.transpose(lrp[:, :4], lsb[:4, :1].to_broadcast([4, P]),
                                idf[:4, :4])
            esb = cpool.tile([P, 4], F32)
            nc.scalar.activation(esb[:], lrp[:, :4], AF.Exp)
            sesb = cpool.tile([P, 1], F32)
            nc.vector.tensor_reduce(out=sesb[:], in_=esb[:],
                                    axis=mybir.AxisListType.X, op=AO.add)
            rse = cpool.tile([P, 1], F32)
            nc.vector.reciprocal(rse[:], sesb[:])
            swrep = cpool.tile([P, 4], F32)
            nc.vector.tensor_scalar_mul(swrep[:], esb[:], rse[:])

            gssw = cpool.tile([P, 4, H], F32)
            btsw = cpool.tile([P, 4, H], F32)
            for b in range(4):
                nc.vector.tensor_scalar_mul(gssw[:, b, :], gsb[:, b, :],
                                            swrep[:, b:b + 1])
                nc.vector.tensor_scalar_mul(btsw[:, b, :], btb[:, b, :],
                                            swrep[:, b:b + 1])

            # ---- d = x_own @ q_b, row-major [128, 3, T] ----
            dall = cpool.tile([P, 3, T], F32)
            for t in range(T):
                dps = psum.tile([P, 6], F32, tag="drp")
                for k in range(2):
                    nc.tensor.matmul(dps[:], lhsT=xt[:, k, t * P:(t + 1) * P],
                                     rhs=pq[:, k, :],
                                     start=(k == 0), stop=(k == 1))
                nc.vector.tensor_copy(out=dall[:, :, t:t + 1],
                                      in_=dps[:, 3:6, None])

            # ---- epilogue helper: z(psum) -> comb ----
            def epilogue(zps, b, comb, first):
                v = epi.tile([P, H], F32, tag="v")
                nc.vector.tensor_tensor(out=v[:], in0=zps[:], in1=cbb[:, b, :],
                                        op=AO.add)
                sm = epi.tile([P, 1], F32, tag="sm")
                nc.vector.tensor_reduce(out=sm[:], in_=v[:],
                                        axis=mybir.AxisListType.X, op=AO.add)
                sq = epi.tile([P, H], F32, tag="sq")
                ss = epi.tile([P, 1], F32, tag="ss")
                nc.scalar.activation(sq[:], v[:], AF.Square, accum_out=ss[:])
                mu = epi.tile([P, 1], F32, tag="mu")
                nc.vector.tensor_scalar_mul(mu[:], sm[:], 1.0 / H)
                m2 = epi.tile([P, 1], F32, tag="m2")
                nc.vector.tensor_scalar_mul(m2[:], ss[:], 1.0 / H)
                mq = epi.tile([P, 1], F32, tag="mq")
                nc.scalar.activation(mq[:], mu[:], AF.Square)
                var = epi.tile([P, 1], F32, tag="var")
                nc.vector.tensor_tensor(out=var[:], in0=m2[:], in1=mq[:],
                                        op=AO.subtract)
                st = epi.tile([P, 1], F32, tag="st")
                nc.scalar.activation(st[:], var[:], AF.Sqrt, bias=epsc[:])
                ri = epi.tile([P, 1], F32, tag="ri")
                nc.vector.reciprocal(ri[:], st[:])
                t1 = epi.tile([P, H], F32, tag="t1")
                nc.vector.tensor_scalar(t1[:], v[:], mu[:], ri[:],
                                        AO.subtract, AO.mult)
                if b < 4:
                    ga, ba = gssw[:, b, :], btsw[:, b, :]
                else:
                    ga, ba = fin[:, 0, :], fin[:, 1, :]
                t2 = epi.tile([P, H], F32, tag="t2")
                nc.vector.tensor_tensor(out=t2[:], in0=t1[:], in1=ga, op=AO.mult)
                t3 = epi.tile([P, H], F32, tag="t3")
                nc.vector.tensor_tensor(out=t3[:], in0=t2[:], in1=ba, op=AO.add)
                if first:
                    nc.scalar.activation(comb[:], t3[:], AF.Relu)
                else:
                    t4 = epi.tile([P, H], F32, tag="t4")
                    nc.scalar.activation(t4[:], t3[:], AF.Relu)
                    nc.vector.tensor_tensor(out=comb[:], in0=comb[:],
                                            in1=t4[:], op=AO.add)

            # ---- main loop over dst tiles ----
            offs = np.concatenate([[0], np.cumsum(cgs)]).astype(int)
            gi = 0
            for t in range(T):
                comb = work.tile([P, H], F32, tag="comb")
                for b in range(3):
                    CG = int(cgs[gi])
                    off = int(offs[gi])
                    gi += 1
                    gt = work.tile([P, CG, GROW], BF16, tag="gt")
                    for cg in range(CG):
                        nc.gpsimd.indirect_dma_start(
                            out=gt[:, cg, :], out_offset=None,
                            in_=gx.ap(),
                            in_offset=bass.IndirectOffsetOnAxis(
                                ap=srcT[:, off + cg:off + cg + 1], axis=0),
                        )
                    # s_src per chunk: s_e = dot(x_src_e, p_b)
                    ssc = work.tile([P, CG], F32, tag="ssc")
                    for cg in range(CG):
                        scr = work.tile([P, H], F32, tag="scr")
                        nc.vector.tensor_tensor(out=scr[:], in0=gt[:, cg, 0:H],
                                                in1=prep[:, b, :], op=AO.mult)
                        nc.vector.tensor_reduce(out=ssc[:, cg:cg + 1],
                                                in_=scr[:],
                                                axis=mybir.AxisListType.X,
                                                op=AO.add)
                    # d replicated across rows
                    drp = psum.tile([P, P], F32, tag="drp")
                    nc.tensor.transpose(drp[:],
                                        dall[:, b, t:t + 1].to_broadcast([P, P]),
                                        idf)
                    dre = work.tile([P, P], F32, tag="dre")
                    nc.vector.tensor_copy(out=dre[:], in_=drp[:])
                    # logits T[e,j] = s_e + d_j ; leaky-relu via max(T, .2T)
                    tw = work.tile([P, CG, P], F32, tag="tw")
                    for cg in range(CG):
                        nc.vector.tensor_scalar_add(tw[:, cg, :], dre[:],
                                                    ssc[:, cg:cg + 1])
                    t02 = work.tile([P, CG, P], F32, tag="t02")
                    nc.vector.tensor_scalar_mul(t02[:], tw[:], NEG_SLOPE)
                    tm = work.tile([P, CG, P], F32, tag="tm")
                    nc.vector.tensor_tensor(out=tm[:], in0=tw[:], in1=t02[:],
                                            op=AO.max)
                    ex = work.tile([P, CG, P], F32, tag="ex")
                    nc.scalar.activation(ex[:], tm[:], AF.Exp)
                    oh = work.tile([P, CG, P], F32, tag="oh")
                    for cg in range(CG):
                        nc.vector.tensor_tensor(
                            out=oh[:, cg, :],
                            in0=dstT[:, off + cg:off + cg + 1].to_broadcast([P, P]),
                            in1=iota, op=AO.is_equal)
                    sm_ = work.tile([P, CG, P], BF16, tag="smat")
                    nc.vector.tensor_tensor(out=sm_[:], in0=ex[:], in1=oh[:],
                                            op=AO.mult)
                    # scatter matmul: num + denom in one PSUM accumulation
                    yps = psy.tile([P, GROW], F32, tag="yps")
                    for cg in range(CG):
                        nc.tensor.matmul(yps[:], lhsT=sm_[:, cg, :],
                                         rhs=gt[:, cg, :],
                                         start=(cg == 0), stop=(cg == CG - 1))
                    rec = work.tile([P, 1], F32, tag="rec")
                    nc.vector.reciprocal(rec[:], yps[:, FLAG:FLAG + 1])
                    y = work.tile([P, H], BF16, tag="y")
                    nc.vector.tensor_scalar_mul(y[:], yps[:, 0:H], rec[:])
                    # z = y @ C_b   (transpose y, then 2 matmuls)
                    yT = work.tile([P, 2, P], BF16, tag="yT")
                    for k in range(2):
                        tps = psum.tile([P, P], BF16, tag="tps")
                        nc.tensor.transpose(tps[:], y[:, k * P:(k + 1) * P], idb)
                        nc.vector.tensor_copy(out=yT[:, k, :], in_=tps[:])
                    zps = psy.tile([P, H], F32, tag="zps")
                    for k in range(2):
                        nc.tensor.matmul(zps[:], lhsT=yT[:, k, :],
                                         rhs=wc[:, b, k, :],
                                         start=(k == 0), stop=(k == 1))
                    epilogue(zps, b, comb, first=(b == 0))
                # branch 3: identity branch
                zps = psy.tile([P, H], F32, tag="zps")
                for k in range(2):
                    nc.tensor.matmul(zps[:], lhsT=xt[:, k, t * P:(t + 1) * P],
                                     rhs=wc[:, 3, k, :],
                                     start=(k == 0), stop=(k == 1))
                epilogue(zps, 3, comb, first=False)
                # fusion: out_t = relu(LN(comb @ Wf + bf))
                cb16 = work.tile([P, H], BF16, tag="cb16")
                nc.vector.tensor_copy(out=cb16[:], in_=comb[:])
                cT = work.tile([P, 2, P], BF16, tag="cT")
                for k in range(2):
                    tps = psum.tile([P, P], BF16, tag="tps")
                    nc.tensor.transpose(tps[:], cb16[:, k * P:(k + 1) * P], idb)
                    nc.vector.tensor_copy(out=cT[:, k, :], in_=tps[:])
                fps = psy.tile([P, H], F32, tag="zps")
                for k in range(2):
                    nc.tensor.matmul(fps[:], lhsT=cT[:, k, :],
                                     rhs=wc[:, 4, k, :],
                                     start=(k == 0), stop=(k == 1))
                # final LN: bias is fin[:,2,:], gain fin[:,0,:], beta fin[:,1,:]
                v = epi.tile([P, H], F32, tag="v")
                nc.vector.tensor_tensor(out=v[:], in0=fps[:], in1=fin[:, 2, :],
                                        op=AO.add)
                sm2 = epi.tile([P, 1], F32, tag="sm")
                nc.vector.tensor_reduce(out=sm2[:], in_=v[:],
                                        axis=mybir.AxisListType.X, op=AO.add)
                sq = epi.tile([P, H], F32, tag="sq")
                ss = epi.tile([P, 1], F32, tag="ss")
                nc.scalar.activation(sq[:], v[:], AF.Square, accum_out=ss[:])
                mu = epi.tile([P, 1], F32, tag="mu")
                nc.vector.tensor_scalar_mul(mu[:], sm2[:], 1.0 / H)
                m2 = epi.tile([P, 1], F32, tag="m2")
                nc.vector.tensor_scalar_mul(m2[:], ss[:], 1.0 / H)
                mq = epi.tile([P, 1], F32, tag="mq")
                nc.scalar.activation(mq[:], mu[:], AF.Square)
                var = epi.tile([P, 1], F32, tag="var")
                nc.vector.tensor_tensor(out=var[:], in0=m2[:], in1=mq[:],
                                        op=AO.subtract)
                st = epi.tile([P, 1], F32, tag="st")
                nc.scalar.activation(st[:], var[:], AF.Sqrt, bias=epsc[:])
                ri = epi.tile([P, 1], F32, tag="ri")
                nc.vector.reciprocal(ri[:], st[:])
                t1 = epi.tile([P, H], F32, tag="t1")
                nc.vector.tensor_scalar(t1[:], v[:], mu[:], ri[:],
                                        AO.subtract, AO.mult)
                t2 = epi.tile([P, H], F32, tag="t2")
                nc.vector.tensor_tensor(out=t2[:], in0=t1[:], in1=fin[:, 0, :],
                                        op=AO.mult)
                t3 = epi.tile([P, H], F32, tag="t3")
                nc.vector.tensor_tensor(out=t3[:], in0=t2[:], in1=fin[:, 1, :],
                                        op=AO.add)
                osb = work.tile([P, H], F32, tag="osb")
                nc.scalar.activation(osb[:], t3[:], AF.Relu)
                nc.sync.dma_start(out=out.ap()[t * P:(t + 1) * P, :], in_=osb[:])

    nc.compile()
    return nc


# --------------------------------------------------------------------------
# host side
# --------------------------------------------------------------------------

def kernel(x, edge_index, edge_attr, Wg, a_src, a_dst, bg, Ws, bs, gs, betas,
           W1, b1, W2, b2, Wf, bf, gf, betaf):
    global LAST_EXEC_NS
    x = np.asarray(x, np.float32)
    N = x.shape[0]
    R = int(np.ceil(N / NCORES / P)) * P
    T = R // P
    NPAD = NCORES * R
    DUMMY_PAD = NPAD
    DUMMY_SELF = NPAD + 1
    GR = int(np.ceil((NPAD + 2) / P)) * P

    Wg = np.asarray(Wg, np.float64)
    a_src_ = np.asarray(a_src, np.float64)
    a_dst_ = np.asarray(a_dst, np.float64)
    bg = np.asarray(bg, np.float64)
    Ws_ = np.asarray(Ws, np.float64)
    bs_ = np.asarray(bs, np.float64)

    p = np.stack([Wg[i] @ a_src_[i] for i in range(3)])
    q = np.stack([Wg[i] @ a_dst_[i] for i in range(3)])
    C = np.stack([Wg[i] @ Ws_[i] for i in range(3)])
    cb = np.stack([bg[i] @ Ws_[i] + bs_[i] for i in range(3)])

    # gather table (replicated)
    gxt = np.zeros((GR, GROW), dtype=ml_dtypes.bfloat16)
    gxt[:N, :H] = _bf(x)
    gxt[:N, FLAG] = 1.0
    gxt[DUMMY_SELF, FLAG] = 1.0

    # ---- edge bucketing ----
    src = np.asarray(edge_index)[0].astype(np.int64)
    dst = np.asarray(edge_index)[1].astype(np.int64)
    attr = np.asarray(edge_attr).astype(np.int64)
    keep = attr < 3
    ks = src[keep]
    kd = dst[keep]
    ka = attr[keep]
    core_of = kd // R
    tl = (kd - core_of * R) // P
    jl = (kd - core_of * R) % P
    # group id = ((core*T + t)*3 + b)
    gid = (core_of * T + tl) * 3 + ka
    order = np.argsort(gid, kind="stable")
    gid_s, ks_s, jl_s = gid[order], ks[order], jl[order]
    counts = np.bincount(gid_s, minlength=NCORES * T * 3).reshape(NCORES, T, 3)
    bounds = np.concatenate([[0], np.cumsum(
        counts.reshape(-1))]).astype(np.int64)

    # chunk counts per (t, b): max over cores, incl. 128 self edges
    cgs = np.ceil((counts.max(axis=0) + P) / P).astype(np.int64)  # [T, 3]
    cgs_tb = cgs.reshape(-1)  # (t, b) order
    totc = int(cgs_tb.sum())
    offs = np.concatenate([[0], np.cumsum(cgs_tb)]).astype(np.int64)

    srcT = np.full((NCORES, P, totc), DUMMY_PAD, dtype=np.int32)
    dstT = np.zeros((NCORES, P, totc), dtype=np.float32)
    selfsrc = np.arange(P, dtype=np.int64)
    for c in range(NCORES):
        for t in range(T):
            for b in range(3):
                g = (c * T + t) * 3 + b
                lo, hi = bounds[g], bounds[g + 1]
                e_src = ks_s[lo:hi]
                e_jl = jl_s[lo:hi]
                gbase = c * R + t * P
                s_self = gbase + selfsrc
                s_self = np.where(s_self < N, s_self, DUMMY_SELF)
                e_src = np.concatenate([e_src, s_self])
                e_jl = np.concatenate([e_jl, selfsrc])
                off = offs[t * 3 + b]
                CG = cgs[t, b]
                ne = len(e_src)
                buf_s = np.full(CG * P, DUMMY_PAD, dtype=np.int64)
                buf_j = np.zeros(CG * P, dtype=np.int64)
                buf_s[:ne] = e_src
                buf_j[:ne] = e_jl
                srcT[c, :, off:off + CG] = buf_s.reshape(CG, P).T
                dstT[c, :, off:off + CG] = buf_j.reshape(CG, P).T

    # ---- constant packs ----
    def rep(v):  # replicate a [H] vector across partitions
        return np.tile(np.asarray(v, np.float32)[None, :], (P, 1))

    wcs = [C[0], C[1], C[2], np.asarray(Ws_[3]), np.asarray(Wf, np.float64)]
    wc = np.zeros((P, 5, 2, H), dtype=ml_dtypes.bfloat16)
    for ci, M in enumerate(wcs):
        for k in range(2):
            wc[:, ci, k, :] = _bf(M[k * P:(k + 1) * P, :])
    prep = np.zeros((P, 3, H), dtype=ml_dtypes.bfloat16)
    for b in range(3):
        prep[:, b, :] = _bf(p[b])[None, :]
    pqa = np.zeros((P, 2, 6), dtype=ml_dtypes.bfloat16)
    for k in range(2):
        for j in range(3):
            pqa[:, k, j] = _bf(p[j][k * P:(k + 1) * P])
            pqa[:, k, 3 + j] = _bf(q[j][k * P:(k + 1) * P])
    gs4 = np.stack([rep(np.asarray(gs)[b]) for b in range(4)], axis=1)
    bt4 = np.stack([rep(np.asarray(betas)[b]) for b in range(4)], axis=1)
    cb4 = np.stack([rep(cb[0]), rep(cb[1]), rep(cb[2]),
                    rep(np.asarray(bs)[3])], axis=1)
    fin = np.stack([rep(gf), rep(betaf), rep(bf)], axis=1)
    w1a = np.zeros((P, 2, P), np.float32)
    W1a = np.asarray(W1, np.float32)
    for k in range(2):
        w1a[:, k, :] = W1a[k * P:(k + 1) * P, :]
    b1a = np.asarray(b1, np.float32).reshape(P, 1)
    w2a = np.asarray(W2, np.float32)  # [128, 4]
    b2a = np.zeros((P, 1), np.float32)
    b2a[:4, 0] = np.asarray(b2, np.float32)
    auxa = np.zeros((P, 2, P), np.float32)
    auxa[:, 0, :] = np.eye(P, dtype=np.float32)
    auxa[:, 1, :] = np.arange(P, dtype=np.float32)[None, :]
    auxb = np.eye(P, dtype=np.float32).astype(ml_dtypes.bfloat16)

    xpad = np.zeros((NPAD, H), np.float32)
    xpad[:N] = x

    key = (T, GR, N, tuple(int(v) for v in cgs_tb))
    if key not in _GRAPH_CACHE:
        _GRAPH_CACHE[key] = _build(T, GR, list(cgs_tb), N)
    nc = _GRAPH_CACHE[key]

    in_maps = []
    for c in range(NCORES):
        xo = xpad[c * R:(c + 1) * R]  # [R, H]
        xtc = np.zeros((P, 2, R), dtype=ml_dtypes.bfloat16)
        xoT = _bf(xo).T  # [H, R]
        for k in range(2):
            xtc[:, k, :] = xoT[k * P:(k + 1) * P, :]
        in_maps.append({
            "gx": gxt,
            "xt": xtc.reshape(P, 2 * R),
            "wc": wc.reshape(P, 5 * 2 * H),
            "prep": prep.reshape(P, 3 * H),
            "pq": pqa.reshape(P, 2 * 6),
            "gsb": gs4.reshape(P, 4 * H),
            "btb": bt4.reshape(P, 4 * H),
            "cbb": cb4.reshape(P, 4 * H),
            "fin": fin.reshape(P, 3 * H),
            "w1": w1a.reshape(P, 2 * P),
            "b1": b1a,
            "w2": w2a,
            "b2": b2a,
            "aux": auxa.reshape(P, 2 * P),
            "auxb": auxb,
            "srcT": srcT[c],
            "dstT": dstT[c],
        })

    trace = os.environ.get("KERNEL_TRACE", "0") == "1"
    res = run_bass_kernel_spmd(nc, in_maps, core_ids=list(range(NCORES)),
                               trace=trace)
    LAST_EXEC_NS = res.exec_time_ns
    global LAST_RES
    LAST_RES = res
    full = np.concatenate([res.results[c]["out"] for c in range(NCORES)],
                          axis=0)
    return full[:N].astype(np.float32)


# revision 12
# speedup vs baseline: 1.2682x; 1.2682x over previous
"""Trainium2 (8 NeuronCores) kernel for nn_AdaptiveSliceSelector.

Strategy:
- Algebraic refold: GAT aggregation commutes with the per-branch weight
  matrix, so per-edge work happens in x-space (256-wide rows) and the
  weight matmuls (Wg@Ws folded into one 256x256 per branch) run on the
  aggregated output. Attention logits use folded vectors p=Wg@a_src,
  q=Wg@a_dst:  s = x@p, d = x@q,  w_e = exp(leaky_relu(s[src]+d[dst])).
  exp() without the max-shift is safe (logits ~ N(0,2)) and cancels in
  the softmax ratio.
- Self-loops are appended as ordinary edges on the host.
- Nodes are sharded round-robin-free: core c owns rows [c*R, (c+1)*R).
  The bf16 gather table (x rows + a validity flag column) is replicated
  to every core's HBM, so cross-partition edges need no halo exchange;
  per-edge rows are fetched with indirect DMA by global src index.
- Per (dst-tile, branch): gathered edge rows [128e, 272] become the
  matmul moving operand; the one-hot scatter matrix S[e, j] =
  w_e * (dstloc_e == j) is built on DVE/ACT and used as lhsT so the
  segment-softmax numerator/denominator come out of one PSUM matmul
  accumulation (the flag column yields the denominator, and host-side
  padding edges point at a zero row so they vanish from both).
- Strategy weights (softmax MLP over mean(x)) use a 1KB AllReduce.
"""

import os
import sys

sys.path.insert(0, "/opt/trn_rl_repo")

import numpy as np
import ml_dtypes

import concourse.bass as bass
import concourse.mybir as mybir
import concourse.tile as tile
from concourse import bacc
from concourse.bass_utils import run_bass_kernel_spmd

NCORES = 8
H = 256
P = 128
GROW = 272  # gather-table row width (bf16) -> 544B rows, 32B aligned
FLAG = H    # column holding the 1.0 validity flag
LN_EPS = 1e-5
NEG_SLOPE = 0.2

F32 = mybir.dt.float32
BF16 = mybir.dt.bfloat16
I32 = mybir.dt.int32
AO = mybir.AluOpType
AF = mybir.ActivationFunctionType

LAST_EXEC_NS = None  # stash for test harness
LAST_RES = None

_GRAPH_CACHE = {}


def _bf(a):
    return np.asarray(a, np.float32).astype(ml_dtypes.bfloat16)


# --------------------------------------------------------------------------
# device graph
# --------------------------------------------------------------------------

def _build(T, GR, cgs, NREAL, flags):
    """Build the SPMD Bass graph.

    T: dst tiles per core; GR: gather table rows; cgs: list of chunk
    counts in (t, b) order, len == 3*T; flags: zero/one-bias fast paths.
    """
    R = T * P
    totc = int(sum(cgs))
    cb_zero, gs_ones, bt_zero, gf_ones, bf_zero, btf_zero = flags

    nc = bacc.Bacc("TRN2", target_bir_lowering=False, debug=False,
                   num_devices=NCORES)

    gx = nc.dram_tensor("gx", [GR, GROW], BF16, kind="ExternalInput")
    xt_in = nc.dram_tensor("xt", [P, 2 * R], BF16, kind="ExternalInput")
    wc_in = nc.dram_tensor("wc", [P, 5 * 2 * H], BF16, kind="ExternalInput")
    prep_in = nc.dram_tensor("prep", [P, 3 * H], BF16, kind="ExternalInput")
    pq_in = nc.dram_tensor("pq", [P, 2 * 6], BF16, kind="ExternalInput")
    gsb_in = nc.dram_tensor("gsb", [P, 4 * H], F32, kind="ExternalInput")
    btb_in = nc.dram_tensor("btb", [P, 4 * H], F32, kind="ExternalInput")
    cbb_in = nc.dram_tensor("cbb", [P, 4 * H], F32, kind="ExternalInput")
    fin_in = nc.dram_tensor("fin", [P, 3 * H], F32, kind="ExternalInput")
    w1_in = nc.dram_tensor("w1", [P, 2 * P], F32, kind="ExternalInput")
    b1_in = nc.dram_tensor("b1", [P, 1], F32, kind="ExternalInput")
    w2_in = nc.dram_tensor("w2", [P, 4], F32, kind="ExternalInput")
    b2_in = nc.dram_tensor("b2", [P, 1], F32, kind="ExternalInput")
    aux_in = nc.dram_tensor("aux", [P, 2 * P], F32, kind="ExternalInput")
    auxb_in = nc.dram_tensor("auxb", [P, 2 * P], BF16, kind="ExternalInput")
    srcT_in = nc.dram_tensor("srcT", [P, totc], I32, kind="ExternalInput")
    dstT_in = nc.dram_tensor("dstT", [P, totc], BF16, kind="ExternalInput")
    xr_in = nc.dram_tensor("xr", [R, GROW], BF16, kind="ExternalInput")
    out = nc.dram_tensor("out", [R, H], F32, kind="ExternalOutput")

    with tile.TileContext(nc) as tc:
        with (
            tc.tile_pool(name="const", bufs=1) as cpool,
            tc.tile_pool(name="work", bufs=3) as work,
            tc.tile_pool(name="epi", bufs=2) as epi,
            tc.tile_pool(name="psum", bufs=2, space="PSUM") as psum,
            tc.tile_pool(name="psy", bufs=2, space="PSUM") as psy,
            tc.tile_pool(name="dram", bufs=1, space="DRAM") as dpool,
        ):
            # ---- resident loads ----
            xt = cpool.tile([P, 2, R], BF16)
            nc.sync.dma_start(out=xt[:], in_=xt_in.ap())
            wc = cpool.tile([P, 5, 2, H], BF16)
            nc.sync.dma_start(out=wc[:], in_=wc_in.ap())
            prepb = []
            for b in range(3):
                pb = cpool.tile([P, H], BF16, tag=f"prep{b}")
                nc.sync.dma_start(out=pb[:], in_=prep_in.ap()[:, b * H:(b + 1) * H])
                prepb.append(pb)
            pq = cpool.tile([P, 2, 6], BF16)
            nc.sync.dma_start(out=pq[:], in_=pq_in.ap())
            gsb = cpool.tile([P, 4, H], F32)
            nc.sync.dma_start(out=gsb[:], in_=gsb_in.ap())
            btb = cpool.tile([P, 4, H], F32)
            nc.sync.dma_start(out=btb[:], in_=btb_in.ap())
            cbb = cpool.tile([P, 4, H], F32)
            nc.sync.dma_start(out=cbb[:], in_=cbb_in.ap())
            fin = cpool.tile([P, 3, H], F32)
            nc.sync.dma_start(out=fin[:], in_=fin_in.ap())
            w1 = cpool.tile([P, 2, P], F32)
            nc.sync.dma_start(out=w1[:], in_=w1_in.ap())
            b1 = cpool.tile([P, 1], F32)
            nc.sync.dma_start(out=b1[:], in_=b1_in.ap())
            w2 = cpool.tile([P, 4], F32)
            nc.sync.dma_start(out=w2[:], in_=w2_in.ap())
            b2 = cpool.tile([P, 1], F32)
            nc.sync.dma_start(out=b2[:], in_=b2_in.ap())
            aux = cpool.tile([P, 2, P], F32)
            nc.sync.dma_start(out=aux[:], in_=aux_in.ap())
            idf = aux[:, 0, :]
            iota = aux[:, 1, :]
            idb = cpool.tile([P, P], BF16)
            nc.sync.dma_start(out=idb[:], in_=auxb_in.ap()[:, 0:P])
            iotab = cpool.tile([P, P], BF16)
            nc.sync.dma_start(out=iotab[:], in_=auxb_in.ap()[:, P:2 * P])
            epsc = cpool.tile([P, 1], F32)
            nc.gpsimd.memset(epsc[:], LN_EPS)
            srcT = cpool.tile([P, totc], I32)
            nc.sync.dma_start(out=srcT[:], in_=srcT_in.ap())
            dstT = cpool.tile([P, totc], BF16)
            nc.sync.dma_start(out=dstT[:], in_=dstT_in.ap())

            # ---- strategy weights: colsum -> AllReduce -> MLP -> sw ----
            cs = cpool.tile([P, 2], F32)
            nc.vector.tensor_reduce(out=cs[:], in_=xt[:],
                                    axis=mybir.AxisListType.X, op=AO.add)
            cin = dpool.tile([P, 2], F32)
            cout = dpool.tile([P, 2], F32)
            nc.gpsimd.dma_start(out=cin[:], in_=cs[:])
            nc.gpsimd.collective_compute(
                "AllReduce", AO.add,
                ins=[cin.opt()], outs=[cout.opt()],
                replica_groups=[list(range(NCORES))],
            )
            gsum = cpool.tile([P, 2], F32)
            nc.gpsimd.dma_start(out=gsum[:], in_=cout[:])
            gmean = cpool.tile([P, 2], F32)
            nc.vector.tensor_scalar_mul(gmean[:], gsum[:], 1.0 / NREAL)

            hps = psum.tile([P, 1], F32, tag="drp")
            for k in range(2):
                nc.tensor.matmul(hps[:], lhsT=w1[:, k, :], rhs=gmean[:, k:k + 1],
                                 start=(k == 0), stop=(k == 1))
            hsb = cpool.tile([P, 1], F32)
            nc.scalar.activation(hsb[:], hps[:], AF.Relu, bias=b1[:])
            lps = psum.tile([P, 4], F32, tag="drp")
            nc.tensor.matmul(lps[:4, :1], lhsT=w2[:], rhs=hsb[:],
                             start=True, stop=True)
            lsb = cpool.tile([P, 1], F32)
            nc.vector.tensor_tensor(out=lsb[:4, :], in0=lps[:4, :1],
                                    in1=b2[:4, :], op=AO.add)
            # transpose+replicate the 4 logits to all partitions
            lrp = psum.tile([P, 4], F32, tag="drp")
            nc.tensor.transpose(lrp[:, :4], lsb[:4, :1].to_broadcast([4, P]),
                                idf[:4, :4])
            esb = cpool.tile([P, 4], F32)
            nc.scalar.activation(esb[:], lrp[:, :4], AF.Exp)
            sesb = cpool.tile([P, 1], F32)
            nc.vector.tensor_reduce(out=sesb[:], in_=esb[:],
                                    axis=mybir.AxisListType.X, op=AO.add)
            rse = cpool.tile([P, 1], F32)
            nc.vector.reciprocal(rse[:], sesb[:])
            swrep = cpool.tile([P, 4], F32)
            nc.vector.tensor_scalar_mul(swrep[:], esb[:], rse[:])

            gssw = cpool.tile([P, 4, H], F32)
            btsw = cpool.tile([P, 4, H], F32)
            for b in range(4):
                nc.vector.tensor_scalar_mul(gssw[:, b, :], gsb[:, b, :],
                                            swrep[:, b:b + 1])
                nc.vector.tensor_scalar_mul(btsw[:, b, :], btb[:, b, :],
                                            swrep[:, b:b + 1])

            # ---- s,d = x_own @ [p|q]_b, row-major [128, 6, T] ----
            dsall = cpool.tile([P, 6, T], F32)
            for t in range(T):
                dps = psum.tile([P, 6], F32, tag="drp")
                for k in range(2):
                    nc.tensor.matmul(dps[:], lhsT=xt[:, k, t * P:(t + 1) * P],
                                     rhs=pq[:, k, :],
                                     start=(k == 0), stop=(k == 1))
                nc.vector.tensor_copy(out=dsall[:, :, t:t + 1],
                                      in_=dps[:, :, None])

            # ---- epilogue A: z(psum) -> v sbuf + per-branch stats ----
            def epi_a(zps, b, vs, sums, sqs):
                if cb_zero[b]:
                    nc.scalar.activation(vs[:, b, :], zps[:], AF.Copy,
                                         accum_out=sums[:, b:b + 1])
                else:
                    nc.vector.tensor_tensor(out=vs[:, b, :], in0=zps[:],
                                            in1=cbb[:, b, :], op=AO.add)
                    nc.vector.tensor_reduce(out=sums[:, b:b + 1],
                                            in_=vs[:, b, :],
                                            axis=mybir.AxisListType.X,
                                            op=AO.add)
                sqd = epi.tile([P, H], F32, tag="sqd")
                nc.scalar.activation(sqd[:], vs[:, b, :], AF.Square,
                                     accum_out=sqs[:, b:b + 1])

            # ---- main loop over dst tiles ----
            offs = np.concatenate([[0], np.cumsum(cgs)]).astype(int)
            gi = 0
            for t in range(T):
                # self-loop rows + weights for this tile (all 3 branches)
                gts = work.tile([P, GROW], BF16, tag="gts")
                nc.sync.dma_start(out=gts[:],
                                  in_=xr_in.ap()[t * P:(t + 1) * P, :])
                ws1 = work.tile([P, 3], F32, tag="ws1")
                nc.vector.tensor_tensor(out=ws1[:], in0=dsall[:, 0:3, t:t + 1],
                                        in1=dsall[:, 3:6, t:t + 1], op=AO.add)
                ws2 = work.tile([P, 3], F32, tag="ws2")
                nc.vector.tensor_scalar_mul(ws2[:], ws1[:], NEG_SLOPE)
                ws3 = work.tile([P, 3], F32, tag="ws3")
                nc.vector.tensor_tensor(out=ws3[:], in0=ws1[:], in1=ws2[:],
                                        op=AO.max)
                wself = work.tile([P, 3], F32, tag="wself")
                nc.scalar.activation(wself[:], ws3[:], AF.Exp)

                vs = epi.tile([P, 4, H], F32, tag="vs")
                sums = epi.tile([P, 4], F32, tag="sums")
                sqs = epi.tile([P, 4], F32, tag="sqs")
                for b in range(3):
                    CG = int(cgs[gi])
                    off = int(offs[gi])
                    gi += 1
                    gt = work.tile([P, CG, GROW], BF16, tag="gt")
                    for cg in range(CG):
                        nc.gpsimd.indirect_dma_start(
                            out=gt[:, cg, :], out_offset=None,
                            in_=gx.ap(),
                            in_offset=bass.IndirectOffsetOnAxis(
                                ap=srcT[:, off + cg:off + cg + 1], axis=0),
                        )
                    # s_src = dot(x_src, p_b)  (batched over chunks)
                    scr = work.tile([P, CG, H], F32, tag="scr")
                    nc.vector.tensor_tensor(
                        out=scr[:], in0=gt[:, :, 0:H],
                        in1=prepb[b][:, None, :].to_broadcast([P, CG, H]),
                        op=AO.mult)
                    ssc = work.tile([P, CG], F32, tag="ssc")
                    nc.vector.tensor_reduce(out=ssc[:], in_=scr[:],
                                            axis=mybir.AxisListType.X,
                                            op=AO.add)
                    # d replicated across rows
                    drp = psum.tile([P, P], F32, tag="drp")
                    nc.tensor.transpose(
                        drp[:], dsall[:, 3 + b, t:t + 1].to_broadcast([P, P]),
                        idf)
                    dre = work.tile([P, P], F32, tag="dre")
                    nc.vector.tensor_copy(out=dre[:], in_=drp[:])
                    # logits T[e,j] = s_e + d_j ; leaky-relu via max(T, .2T)
                    tw = work.tile([P, CG, P], F32, tag="tw")
                    nc.vector.tensor_tensor(
                        out=tw[:],
                        in0=dre[:, None, :].to_broadcast([P, CG, P]),
                        in1=ssc[:, :, None].to_broadcast([P, CG, P]),
                        op=AO.add)
                    t02 = work.tile([P, CG, P], F32, tag="t02")
                    nc.vector.tensor_scalar_mul(t02[:], tw[:], NEG_SLOPE)
                    tm = work.tile([P, CG, P], F32, tag="tm")
                    nc.vector.tensor_tensor(out=tm[:], in0=tw[:], in1=t02[:],
                                            op=AO.max)
                    ex = work.tile([P, CG, P], F32, tag="ex")
                    nc.scalar.activation(ex[:], tm[:], AF.Exp)
                    oh = work.tile([P, CG, P], BF16, tag="oh")
                    nc.vector.tensor_tensor(
                        out=oh[:],
                        in0=dstT[:, off:off + CG, None].to_broadcast(
                            [P, CG, P]),
                        in1=iotab[:, None, :].to_broadcast([P, CG, P]),
                        op=AO.is_equal)
                    sm_ = work.tile([P, CG, P], BF16, tag="smat")
                    nc.vector.tensor_tensor(out=sm_[:], in0=ex[:], in1=oh[:],
                                            op=AO.mult)
                    ssf = work.tile([P, P], BF16, tag="ssf")
                    nc.vector.tensor_scalar_mul(ssf[:], idb,
                                                wself[:, b:b + 1])
                    # scatter matmul: num + denom in one PSUM accumulation
                    yps = psy.tile([P, GROW], F32, tag="yps")
                    for cg in range(CG):
                        nc.tensor.matmul(yps[:], lhsT=sm_[:, cg, :],
                                         rhs=gt[:, cg, :],
                                         start=(cg == 0), stop=False)
                    nc.tensor.matmul(yps[:], lhsT=ssf[:], rhs=gts[:],
                                     start=False, stop=True)
                    rec = work.tile([P, 1], F32, tag="rec")
                    nc.vector.reciprocal(rec[:], yps[:, FLAG:FLAG + 1])
                    y = work.tile([P, H], BF16, tag="y")
                    nc.vector.tensor_scalar_mul(y[:], yps[:, 0:H], rec[:])
                    # z = y @ C_b   (transpose y, then 2 matmuls)
                    yT = work.tile([P, 2, P], BF16, tag="yT")
                    for k in range(2):
                        tps = psum.tile([P, P], BF16, tag="tps")
                        nc.tensor.transpose(tps[:], y[:, k * P:(k + 1) * P],
                                            idb)
                        nc.vector.tensor_copy(out=yT[:, k, :], in_=tps[:])
                    zps = psy.tile([P, H], F32, tag="zps")
                    for k in range(2):
                        nc.tensor.matmul(zps[:], lhsT=yT[:, k, :],
                                         rhs=wc[:, b, k, :],
                                         start=(k == 0), stop=(k == 1))
                    epi_a(zps, b, vs, sums, sqs)
                # branch 3: identity branch
                zps = psy.tile([P, H], F32, tag="zps")
                for k in range(2):
                    nc.tensor.matmul(zps[:], lhsT=xt[:, k, t * P:(t + 1) * P],
                                     rhs=wc[:, 3, k, :],
                                     start=(k == 0), stop=(k == 1))
                epi_a(zps, 3, vs, sums, sqs)
                # batched LN stats for the 4 branches
                mus = epi.tile([P, 4], F32, tag="mus")
                nc.vector.tensor_scalar_mul(mus[:], sums[:], 1.0 / H)
                m2s = epi.tile([P, 4], F32, tag="m2s")
                nc.vector.tensor_scalar_mul(m2s[:], sqs[:], 1.0 / H)
                mqs = epi.tile([P, 4], F32, tag="mqs")
                nc.scalar.activation(mqs[:], mus[:], AF.Square)
                vrs = epi.tile([P, 4], F32, tag="vrs")
                nc.vector.tensor_tensor(out=vrs[:], in0=m2s[:], in1=mqs[:],
                                        op=AO.subtract)
                sts = epi.tile([P, 4], F32, tag="sts")
                nc.scalar.activation(sts[:], vrs[:], AF.Sqrt, bias=epsc[:])
                ris = epi.tile([P, 4], F32, tag="ris")
                nc.vector.reciprocal(ris[:], sts[:])
                rsw = epi.tile([P, 4], F32, tag="rsw")
                nc.vector.tensor_tensor(out=rsw[:], in0=ris[:], in1=swrep[:],
                                        op=AO.mult)
                comb = work.tile([P, H], F32, tag="comb")
                for b in range(4):
                    sc2 = rsw if gs_ones[b] else ris
                    t1 = epi.tile([P, H], F32, tag="t1")
                    nc.vector.tensor_scalar(t1[:], vs[:, b, :],
                                            mus[:, b:b + 1], sc2[:, b:b + 1],
                                            AO.subtract, AO.mult)
                    cur = t1
                    if not gs_ones[b]:
                        t2 = epi.tile([P, H], F32, tag="t2")
                        nc.vector.tensor_tensor(out=t2[:], in0=cur[:],
                                                in1=gssw[:, b, :], op=AO.mult)
                        cur = t2
                    if not bt_zero[b]:
                        t3 = epi.tile([P, H], F32, tag="t3")
                        nc.vector.tensor_tensor(out=t3[:], in0=cur[:],
                                                in1=btsw[:, b, :], op=AO.add)
                        cur = t3
                    if b == 0:
                        nc.scalar.activation(comb[:], cur[:], AF.Relu)
                    else:
                        t4 = epi.tile([P, H], F32, tag="t4")
                        nc.scalar.activation(t4[:], cur[:], AF.Relu)
                        nc.vector.tensor_tensor(out=comb[:], in0=comb[:],
                                                in1=t4[:], op=AO.add)
                # fusion: out_t = relu(LN(comb @ Wf + bf))
                cb16 = work.tile([P, H], BF16, tag="cb16")
                nc.vector.tensor_copy(out=cb16[:], in_=comb[:])
                cT = work.tile([P, 2, P], BF16, tag="cT")
                for k in range(2):
                    tps = psum.tile([P, P], BF16, tag="tps")
                    nc.tensor.transpose(tps[:], cb16[:, k * P:(k + 1) * P],
                                        idb)
                    nc.vector.tensor_copy(out=cT[:, k, :], in_=tps[:])
                fps = psy.tile([P, H], F32, tag="zps")
                for k in range(2):
                    nc.tensor.matmul(fps[:], lhsT=cT[:, k, :],
                                     rhs=wc[:, 4, k, :],
                                     start=(k == 0), stop=(k == 1))
                # final LN
                fv = epi.tile([P, H], F32, tag="fv")
                fsum = epi.tile([P, 1], F32, tag="fsum")
                if bf_zero:
                    nc.scalar.activation(fv[:], fps[:], AF.Copy,
                                         accum_out=fsum[:])
                else:
                    nc.vector.tensor_tensor(out=fv[:], in0=fps[:],
                                            in1=fin[:, 2, :], op=AO.add)
                    nc.vector.tensor_reduce(out=fsum[:], in_=fv[:],
                                            axis=mybir.AxisListType.X,
                                            op=AO.add)
                fss = epi.tile([P, 1], F32, tag="fss")
                sqd2 = epi.tile([P, H], F32, tag="sqd")
                nc.scalar.activation(sqd2[:], fv[:], AF.Square,
                                     accum_out=fss[:])
                fmu = epi.tile([P, 1], F32, tag="fmu")
                nc.vector.tensor_scalar_mul(fmu[:], fsum[:], 1.0 / H)
                fm2 = epi.tile([P, 1], F32, tag="fm2")
                nc.vector.tensor_scalar_mul(fm2[:], fss[:], 1.0 / H)
                fmq = epi.tile([P, 1], F32, tag="fmq")
                nc.scalar.activation(fmq[:], fmu[:], AF.Square)
                fvr = epi.tile([P, 1], F32, tag="fvr")
                nc.vector.tensor_tensor(out=fvr[:], in0=fm2[:], in1=fmq[:],
                                        op=AO.subtract)
                fst = epi.tile([P, 1], F32, tag="fst")
                nc.scalar.activation(fst[:], fvr[:], AF.Sqrt, bias=epsc[:])
                fri = epi.tile([P, 1], F32, tag="fri")
                nc.vector.reciprocal(fri[:], fst[:])
                ft1 = epi.tile([P, H], F32, tag="ft1")
                nc.vector.tensor_scalar(ft1[:], fv[:], fmu[:], fri[:],
                                        AO.subtract, AO.mult)
                fcur = ft1
                if not gf_ones:
                    ft2 = epi.tile([P, H], F32, tag="ft2")
                    nc.vector.tensor_tensor(out=ft2[:], in0=fcur[:],
                                            in1=fin[:, 0, :], op=AO.mult)
                    fcur = ft2
                if not btf_zero:
                    ft3 = epi.tile([P, H], F32, tag="ft3")
                    nc.vector.tensor_tensor(out=ft3[:], in0=fcur[:],
                                            in1=fin[:, 1, :], op=AO.add)
                    fcur = ft3
                osb = work.tile([P, H], F32, tag="osb")
                nc.scalar.activation(osb[:], fcur[:], AF.Relu)
                nc.sync.dma_start(out=out.ap()[t * P:(t + 1) * P, :],
                                  in_=osb[:])

    nc.compile()
    return nc


# --------------------------------------------------------------------------
# host side
# --------------------------------------------------------------------------

def kernel(x, edge_index, edge_attr, Wg, a_src, a_dst, bg, Ws, bs, gs, betas,
           W1, b1, W2, b2, Wf, bf, gf, betaf):
    global LAST_EXEC_NS
    x = np.asarray(x, np.float32)
    N = x.shape[0]
    R = int(np.ceil(N / NCORES / P)) * P
    T = R // P
    NPAD = NCORES * R
    DUMMY_PAD = NPAD
    DUMMY_SELF = NPAD + 1
    GR = int(np.ceil((NPAD + 2) / P)) * P

    Wg = np.asarray(Wg, np.float64)
    a_src_ = np.asarray(a_src, np.float64)
    a_dst_ = np.asarray(a_dst, np.float64)
    bg = np.asarray(bg, np.float64)
    Ws_ = np.asarray(Ws, np.float64)
    bs_ = np.asarray(bs, np.float64)

    p = np.stack([Wg[i] @ a_src_[i] for i in range(3)])
    q = np.stack([Wg[i] @ a_dst_[i] for i in range(3)])
    C = np.stack([Wg[i] @ Ws_[i] for i in range(3)])
    cb = np.stack([bg[i] @ Ws_[i] + bs_[i] for i in range(3)])

    # gather table (replicated)
    gxt = np.zeros((GR, GROW), dtype=ml_dtypes.bfloat16)
    gxt[:N, :H] = _bf(x)
    gxt[:N, FLAG] = 1.0
    gxt[DUMMY_SELF, FLAG] = 1.0

    # ---- edge bucketing ----
    src = np.asarray(edge_index)[0].astype(np.int64)
    dst = np.asarray(edge_index)[1].astype(np.int64)
    attr = np.asarray(edge_attr).astype(np.int64)
    keep = attr < 3
    ks = src[keep]
    kd = dst[keep]
    ka = attr[keep]
    core_of = kd // R
    tl = (kd - core_of * R) // P
    jl = (kd - core_of * R) % P
    # group id = ((core*T + t)*3 + b)
    gid = (core_of * T + tl) * 3 + ka
    order = np.argsort(gid, kind="stable")
    gid_s, ks_s, jl_s = gid[order], ks[order], jl[order]
    counts = np.bincount(gid_s, minlength=NCORES * T * 3).reshape(NCORES, T, 3)
    bounds = np.concatenate([[0], np.cumsum(
        counts.reshape(-1))]).astype(np.int64)

    # chunk counts per (t, b): max over cores (self-loops go direct)
    cgs = np.maximum(np.ceil(counts.max(axis=0) / P), 1).astype(np.int64)
    cgs_tb = cgs.reshape(-1)  # (t, b) order
    totc = int(cgs_tb.sum())
    offs = np.concatenate([[0], np.cumsum(cgs_tb)]).astype(np.int64)

    srcT = np.full((NCORES, P, totc), DUMMY_PAD, dtype=np.int32)
    dstT = np.zeros((NCORES, P, totc), dtype=np.float32)
    for c in range(NCORES):
        for t in range(T):
            for b in range(3):
                g = (c * T + t) * 3 + b
                lo, hi = bounds[g], bounds[g + 1]
                e_src = ks_s[lo:hi]
                e_jl = jl_s[lo:hi]
                off = offs[t * 3 + b]
                CG = cgs[t, b]
                ne = len(e_src)
                buf_s = np.full(CG * P, DUMMY_PAD, dtype=np.int64)
                buf_j = np.zeros(CG * P, dtype=np.int64)
                buf_s[:ne] = e_src
                buf_j[:ne] = e_jl
                srcT[c, :, off:off + CG] = buf_s.reshape(CG, P).T
                dstT[c, :, off:off + CG] = buf_j.reshape(CG, P).T

    # ---- constant packs ----
    def rep(v):  # replicate a [H] vector across partitions
        return np.tile(np.asarray(v, np.float32)[None, :], (P, 1))

    wcs = [C[0], C[1], C[2], np.asarray(Ws_[3]), np.asarray(Wf, np.float64)]
    wc = np.zeros((P, 5, 2, H), dtype=ml_dtypes.bfloat16)
    for ci, M in enumerate(wcs):
        for k in range(2):
            wc[:, ci, k, :] = _bf(M[k * P:(k + 1) * P, :])
    prep = np.zeros((P, 3, H), dtype=ml_dtypes.bfloat16)
    for b in range(3):
        prep[:, b, :] = _bf(p[b])[None, :]
    pqa = np.zeros((P, 2, 6), dtype=ml_dtypes.bfloat16)
    for k in range(2):
        for j in range(3):
            pqa[:, k, j] = _bf(p[j][k * P:(k + 1) * P])
            pqa[:, k, 3 + j] = _bf(q[j][k * P:(k + 1) * P])
    gs4 = np.stack([rep(np.asarray(gs)[b]) for b in range(4)], axis=1)
    bt4 = np.stack([rep(np.asarray(betas)[b]) for b in range(4)], axis=1)
    cb4 = np.stack([rep(cb[0]), rep(cb[1]), rep(cb[2]),
                    rep(np.asarray(bs)[3])], axis=1)
    fin = np.stack([rep(gf), rep(betaf), rep(bf)], axis=1)
    w1a = np.zeros((P, 2, P), np.float32)
    W1a = np.asarray(W1, np.float32)
    for k in range(2):
        w1a[:, k, :] = W1a[k * P:(k + 1) * P, :]
    b1a = np.asarray(b1, np.float32).reshape(P, 1)
    w2a = np.asarray(W2, np.float32)  # [128, 4]
    b2a = np.zeros((P, 1), np.float32)
    b2a[:4, 0] = np.asarray(b2, np.float32)
    auxa = np.zeros((P, 2, P), np.float32)
    auxa[:, 0, :] = np.eye(P, dtype=np.float32)
    auxa[:, 1, :] = np.arange(P, dtype=np.float32)[None, :]
    auxb = np.zeros((P, 2, P), dtype=ml_dtypes.bfloat16)
    auxb[:, 0, :] = np.eye(P, dtype=np.float32)
    auxb[:, 1, :] = np.arange(P, dtype=np.float32)[None, :]

    xpad = np.zeros((NPAD, H), np.float32)
    xpad[:N] = x

    cb_zero = tuple(bool(np.all(cb4[:, b, :] == 0)) for b in range(4))
    gs_ones = tuple(bool(np.all(gs4[:, b, :] == 1)) for b in range(4))
    bt_zero = tuple(bool(np.all(bt4[:, b, :] == 0)) for b in range(4))
    gf_ones = bool(np.all(fin[:, 0, :] == 1))
    btf_zero = bool(np.all(fin[:, 1, :] == 0))
    bf_zero = bool(np.all(fin[:, 2, :] == 0))
    flags = (cb_zero, gs_ones, bt_zero, gf_ones, bf_zero, btf_zero)

    key = (T, GR, N, tuple(int(v) for v in cgs_tb), flags)
    if key not in _GRAPH_CACHE:
        _GRAPH_CACHE[key] = _build(T, GR, list(cgs_tb), N, flags)
    nc = _GRAPH_CACHE[key]

    in_maps = []
    for c in range(NCORES):
        xo = xpad[c * R:(c + 1) * R]  # [R, H]
        xtc = np.zeros((P, 2, R), dtype=ml_dtypes.bfloat16)
        xoT = _bf(xo).T  # [H, R]
        for k in range(2):
            xtc[:, k, :] = xoT[k * P:(k + 1) * P, :]
        xrc = gxt[c * R:(c + 1) * R].copy()
        xrc[:, FLAG] = 1.0
        in_maps.append({
            "gx": gxt,
            "xr": xrc,
            "xt": xtc.reshape(P, 2 * R),
            "wc": wc.reshape(P, 5 * 2 * H),
            "prep": prep.reshape(P, 3 * H),
            "pq": pqa.reshape(P, 2 * 6),
            "gsb": gs4.reshape(P, 4 * H),
            "btb": bt4.reshape(P, 4 * H),
            "cbb": cb4.reshape(P, 4 * H),
            "fin": fin.reshape(P, 3 * H),
            "w1": w1a.reshape(P, 2 * P),
            "b1": b1a,
            "w2": w2a,
            "b2": b2a,
            "aux": auxa.reshape(P, 2 * P),
            "auxb": auxb.reshape(P, 2 * P),
            "srcT": srcT[c],
            "dstT": dstT[c].astype(ml_dtypes.bfloat16),
        })

    trace = os.environ.get("KERNEL_TRACE", "0") == "1"
    res = run_bass_kernel_spmd(nc, in_maps, core_ids=list(range(NCORES)),
                               trace=trace)
    LAST_EXEC_NS = res.exec_time_ns
    global LAST_RES
    LAST_RES = res
    full = np.concatenate([res.results[c]["out"] for c in range(NCORES)],
                          axis=0)
    return full[:N].astype(np.float32)
